# revision 1
# baseline (speedup 1.0000x reference)
"""BiMambaBlock Trainium2 kernel (8-core SPMD).

Sharding: core c -> (seq = c//2, half = c%2) where seq in
{b0 fwd, b0 bwd, b1 fwd, b1 bwd} and half selects 256 of the 512 d_inner
channels.  The selective scan runs as 16 per-state-index first-order
recurrences via the DVE tensor_tensor_scan instruction (time in the free
dimension, channels in partitions).  Cross-core reductions: a pair
AllReduce for the x-projection partial sums and a quad ReduceScatter for
the mamba output; the FFN + layernorm run token-parallel after the
scatter.
"""
import sys

for _p in ("/opt/trn_rl_repo",):
    if _p not in sys.path:
        sys.path.insert(0, _p)

import numpy as np
import ml_dtypes
from contextlib import ExitStack

import concourse.bass as bass
import concourse.tile as tile
from concourse import bacc, mybir
from concourse.bass_utils import run_bass_kernel_spmd
from concourse.masks import make_identity

BF = ml_dtypes.bfloat16
FP32 = mybir.dt.float32
BF16 = mybir.dt.bfloat16
AF = mybir.ActivationFunctionType
OP = mybir.AluOpType

# problem constants (hardcoded per contract)
B, L, DM = 2, 4096, 256
DI_FULL = 512
DI = 256           # d_inner half per core
NS = 16            # d_state
RK = 16            # dt_rank
DC = 4             # conv width
H_FF = 1024        # ffn hidden
T_SLAB = 1024      # tokens per core in ffn phase
NQ = 4             # scan processed in NQ time chunks
LQ = L // NQ       # 1024
LN_EPS = 1e-5
PJ = RK + 2 * NS   # 48

_NC_CACHE = {}


def _inp(nc, name, shape, dtype):
    return nc.dram_tensor(name, shape, dtype, kind="ExternalInput").ap()


def _out(nc, name, shape, dtype):
    return nc.dram_tensor(name, shape, dtype, kind="ExternalOutput").ap()


def _bcast_row(row_ap, parts=128):
    """AP reading a single (1, N) row replicated across `parts` partitions."""
    return bass.AP(tensor=row_ap.tensor, offset=row_ap.offset,
                   ap=[[0, parts]] + row_ap.ap[1:])


def build_program(reps=1):
    nc = bacc.Bacc("TRN2", target_bir_lowering=False, debug=False, num_devices=8)

    # ---- inputs (per-core data prepared on host) ----
    xT = _inp(nc, "xT", [DM, L], BF16)                # x (transposed, flipped if bwd)
    w_xs = _inp(nc, "w_xs", [DM, DI], BF16)           # W_in xs cols for this half
    w_z = _inp(nc, "w_z", [DM, DI], BF16)             # W_in z cols
    conv_w = _inp(nc, "conv_w", [DI, DC], FP32)
    conv_b = _inp(nc, "conv_b", [DI, 1], FP32)
    w_xp = _inp(nc, "w_xp", [DI, PJ], BF16)           # (256, 48)
    w_dt = _inp(nc, "w_dt", [RK, DI], BF16)           # (16, 256)
    b_dt = _inp(nc, "b_dt", [DI, 1], FP32)
    a_mat = _inp(nc, "a_mat", [DI, NS], FP32)         # A = -exp(A_log) half
    dp = _inp(nc, "dp", [DI, 1], FP32)
    w_out = _inp(nc, "w_out", [DI, DM], BF16)         # (256, 256)
    w_ff1 = _inp(nc, "w_ff1", [DM, H_FF], BF16)
    b_ff1 = _inp(nc, "b_ff1", [128, H_FF // 128], FP32)   # col j = bias block j
    w_ff2 = _inp(nc, "w_ff2", [H_FF, DM], BF16)
    b_ff2r = _inp(nc, "b_ff2r", [128, DM], FP32)      # row-broadcast bias
    g2r = _inp(nc, "g2r", [128, DM], FP32)
    beta2r = _inp(nc, "beta2r", [128, DM], FP32)
    sel = _inp(nc, "sel", [128, 2], FP32)             # [fwd?, bwd?] per core

    out_slab = _out(nc, "out_slab", [T_SLAB, DM], FP32)

    # ---- internal DRAM ----
    proj_dram = nc.dram_tensor("proj_dram", [PJ, L], FP32)
    proj_ar = nc.dram_tensor("proj_ar", [PJ, L], FP32)
    bc_dram = nc.dram_tensor("bc_dram", [2 * NS, L], BF16)
    rs_in = nc.dram_tensor("rs_in", [L, DM], FP32)
    rs_out = nc.dram_tensor("rs_out", [T_SLAB, DM], FP32)

    NB = DI // 128  # = 2 channel blocks per core
    NJ = L // 512

    with tile.TileContext(nc) as tc, ExitStack() as ctx:
        consts = ctx.enter_context(tc.tile_pool(name="consts", bufs=1))
        persist = ctx.enter_context(tc.tile_pool(name="persist", bufs=1))
        work = ctx.enter_context(tc.tile_pool(name="work", bufs=1))
        scanp = ctx.enter_context(tc.tile_pool(name="scanp", bufs=3))
        psum = ctx.enter_context(tc.tile_pool(name="psum", bufs=4, space="PSUM"))
        psumf = ctx.enter_context(tc.tile_pool(name="psumf", bufs=2, space="PSUM"))

        # ---------- constants ----------
        def cload(shape, dtype, src, nm):
            t = consts.tile(shape, dtype, tag=nm, name=nm)
            nc.sync.dma_start(out=t, in_=src)
            return t

        w_xs_sb = [cload([128, DI], BF16, w_xs[k * 128:(k + 1) * 128, :], f"wxs{k}")
                   for k in range(2)]
        w_z_sb = [cload([128, DI], BF16, w_z[k * 128:(k + 1) * 128, :], f"wz{k}")
                  for k in range(2)]
        conv_w_sb = [cload([128, DC], FP32, conv_w[b * 128:(b + 1) * 128, :], f"cw{b}")
                     for b in range(NB)]
        conv_b_sb = [cload([128, 1], FP32, conv_b[b * 128:(b + 1) * 128, :], f"cb{b}")
                     for b in range(NB)]
        b_dt_sb = [cload([128, 1], FP32, b_dt[b * 128:(b + 1) * 128, :], f"bdt{b}")
                   for b in range(NB)]
        a_sb = [cload([128, NS], FP32, a_mat[b * 128:(b + 1) * 128, :], f"am{b}")
                for b in range(NB)]
        dp_sb = [cload([128, 1], FP32, dp[b * 128:(b + 1) * 128, :], f"dp{b}")
                 for b in range(NB)]
        w_xp_sb = [cload([128, PJ], BF16, w_xp[b * 128:(b + 1) * 128, :], f"wxp{b}")
                   for b in range(NB)]
        w_dt_sb = cload([RK, DI], BF16, w_dt[:, :], "wdt")
        w_out_sb = [cload([128, DM], BF16, w_out[b * 128:(b + 1) * 128, :], f"wo{b}")
                    for b in range(NB)]
        sel_sb = cload([128, 2], FP32, sel[:, :], "selc")
        w_ff1_sb = [cload([128, H_FF], BF16, w_ff1[k * 128:(k + 1) * 128, :], f"wf1{k}")
                    for k in range(2)]
        w_ff2_sb = [cload([128, DM], BF16, w_ff2[m * 128:(m + 1) * 128, :], f"wf2{m}")
                    for m in range(8)]
        b_ff1_sb = cload([128, H_FF // 128], FP32, b_ff1[:, :], "bf1")
        b_ff2_sb = cload([128, DM], FP32, b_ff2r[:, :], "bf2")
        g2_sb = cload([128, DM], FP32, g2r[:, :], "g2c")
        beta2_sb = cload([128, DM], FP32, beta2r[:, :], "be2")
        eps_sb = consts.tile([128, 1], FP32, tag="eps", name="eps")
        nc.vector.memset(eps_sb, LN_EPS)
        identity_bf = consts.tile([128, 128], FP32, tag="idn", name="idn")
        make_identity(nc, identity_bf)

        for _rep in range(reps):
            # big shared slabs: xT reused later by the ffn gelu activations
            big = [persist.tile([128, L], BF16, tag=f"big{b}", name=f"big{b}")
                   for b in range(NB)]
            for b in range(NB):
                nc.sync.dma_start(out=big[b], in_=xT[b * 128:(b + 1) * 128, :])
            xT_sb = big

            # ---------- phase 1: xs / z matmuls, conv, silu ----------
            xs_pad = [persist.tile([128, DC - 1 + L], BF16, tag=f"xsp{b}", name=f"xsp{b}")
                      for b in range(NB)]
            zsil = [persist.tile([128, L], BF16, tag=f"zs{b}", name=f"zs{b}")
                    for b in range(NB)]
            for b in range(NB):
                nc.vector.memset(xs_pad[b][:, 0:DC - 1], 0.0)
            for b in range(NB):
                msl = slice(b * 128, (b + 1) * 128)
                for j in range(NJ):
                    jsl = slice(j * 512, (j + 1) * 512)
                    ps = psum.tile([128, 512], FP32, tag="mm", name="mm")
                    for k in range(2):
                        nc.tensor.matmul(ps, w_xs_sb[k][:, msl], xT_sb[k][:, jsl],
                                         start=(k == 0), stop=(k == 1))
                    nc.scalar.copy(xs_pad[b][:, DC - 1 + j * 512:DC - 1 + (j + 1) * 512], ps)
                    ps2 = psum.tile([128, 512], FP32, tag="mm", name="mm")
                    for k in range(2):
                        nc.tensor.matmul(ps2, w_z_sb[k][:, msl], xT_sb[k][:, jsl],
                                         start=(k == 0), stop=(k == 1))
                    nc.scalar.activation(zsil[b][:, jsl], ps2, AF.Silu)

            # causal depthwise conv + silu -> xc (per time-quarter, on gpsimd)
            xc = [persist.tile([128, L], BF16, tag=f"xc{b}", name=f"xc{b}")
                  for b in range(NB)]
            for b in range(NB):
                for q in range(NQ):
                    acc = work.tile([128, LQ], FP32, tag="convacc", name="convacc", bufs=1)
                    base = q * LQ
                    nc.vector.tensor_scalar(acc, xs_pad[b][:, base:base + LQ],
                                            conv_w_sb[b][:, 0:1], None, OP.mult)
                    for k in range(1, DC):
                        nc.vector.scalar_tensor_tensor(
                            acc, xs_pad[b][:, base + k:base + k + LQ],
                            conv_w_sb[b][:, k:k + 1], acc, OP.mult, OP.add)
                    nc.scalar.activation(xc[b][:, base:base + LQ], acc, AF.Silu,
                                         bias=conv_b_sb[b][:, 0:1])

            # ---------- phase 2: xproj matmul, pair AllReduce, delta ----------
            for j in range(NJ):
                jsl = slice(j * 512, (j + 1) * 512)
                ps = psum.tile([128, 512], FP32, tag="mm", name="mm")[0:PJ, :]
                for b in range(NB):
                    nc.tensor.matmul(ps, w_xp_sb[b], xc[b][:, jsl],
                                     start=(b == 0), stop=(b == NB - 1))
                pj = work.tile([PJ, 512], FP32, tag="pjchunk", name="pjchunk", bufs=2)
                nc.scalar.copy(pj, ps)
                nc.sync.dma_start(out=proj_dram[:, jsl], in_=pj)
            nc.gpsimd.collective_compute(
                "AllReduce", OP.add,
                replica_groups=[[0, 1], [2, 3], [4, 5], [6, 7]],
                ins=[proj_dram[:, :]], outs=[proj_ar[:, :]])
            projT_bf = persist.tile([PJ, L], BF16, tag="projbf", name="projbf")
            for j in range(NJ):
                jsl = slice(j * 512, (j + 1) * 512)
                pj = work.tile([PJ, 512], FP32, tag="pjchunk", name="pjchunk", bufs=2)
                nc.sync.dma_start(out=pj, in_=proj_ar[:, jsl])
                nc.scalar.copy(projT_bf[:, jsl], pj)
            # bf16 B/C rows staged in DRAM so the scan can partition-broadcast them
            nc.sync.dma_start(out=bc_dram[:, :], in_=projT_bf[RK:PJ, :])

            # delta = softplus(dt @ W_dt + b_dt) = ln(exp(u + b) + 1) : (DI, L) bf16
            delta = [persist.tile([128, L], BF16, tag=f"dl{b}", name=f"dl{b}")
                     for b in range(NB)]
            for b in range(NB):
                msl = slice(b * 128, (b + 1) * 128)
                for j in range(NJ):
                    jsl = slice(j * 512, (j + 1) * 512)
                    ps = psum.tile([128, 512], FP32, tag="mm", name="mm")
                    nc.tensor.matmul(ps, w_dt_sb[:, msl], projT_bf[0:RK, jsl],
                                     start=True, stop=True)
                    eu = work.tile([128, 512], FP32, tag="softp", name="softp", bufs=1)
                    nc.scalar.activation(eu, ps, AF.Exp, bias=b_dt_sb[b][:, 0:1])
                    nc.scalar.activation(delta[b][:, jsl], eu, AF.Ln, bias=1.0)

            # ---------- phase 3: selective scan (time quarters) ----------
            NLANES = 4
            yfin = [persist.tile([128, L], BF16, tag=f"yf{b}", name=f"yf{b}")
                    for b in range(NB)]
            h_last = persist.tile([128, NS * NB], FP32, tag="hl", name="hl")
            for q in range(NQ):
                qsl = slice(q * LQ, (q + 1) * LQ)
                # dxq = delta * xc for this quarter (gpsimd)
                dxq = []
                for b in range(NB):
                    d = work.tile([128, LQ], BF16, tag=f"dxq{b}", name=f"dxq{b}", bufs=2)
                    nc.gpsimd.tensor_tensor(d, delta[b][:, qsl], xc[b][:, qsl], OP.mult)
                    dxq.append(d)
                lanes = [[work.tile([128, LQ], BF16, tag=f"lane{ln}_{b}",
                                    name=f"lane{ln}_{b}", bufs=1)
                          for ln in range(NLANES)] for b in range(NB)]
                for n in range(NS):
                    b_rep = scanp.tile([128, LQ], BF16, tag="brep", name="brep")
                    nc.sync.dma_start(
                        out=b_rep, in_=_bcast_row(bc_dram[n:n + 1, qsl]))
                    c_rep = scanp.tile([128, LQ], BF16, tag="crep", name="crep")
                    nc.sync.dma_start(
                        out=c_rep, in_=_bcast_row(bc_dram[NS + n:NS + n + 1, qsl]))
                    for b in range(NB):
                        hcol = NS * b + n
                        dA = scanp.tile([128, LQ], BF16, tag="dA", name="dA")
                        nc.scalar.activation(dA, delta[b][:, qsl], AF.Exp,
                                             scale=a_sb[b][:, n:n + 1])
                        dBx = scanp.tile([128, LQ], BF16, tag="dBx", name="dBx")
                        nc.gpsimd.tensor_tensor(dBx, dxq[b], b_rep, OP.mult)
                        h = scanp.tile([128, LQ], BF16, tag="h", name="h")
                        init = 0.0 if q == 0 else h_last[:, hcol:hcol + 1]
                        nc.vector.tensor_tensor_scan(h, dA, dBx, init, OP.mult, OP.add)
                        if q < NQ - 1:
                            nc.vector.tensor_copy(h_last[:, hcol:hcol + 1],
                                                  h[:, LQ - 1:LQ])
                        hc = scanp.tile([128, LQ], BF16, tag="dBx", name="hc")
                        nc.vector.tensor_tensor(hc, h, c_rep, OP.mult)
                        # accumulate into bf16 lane via gpsimd software-DGE DMA
                        nc.gpsimd.dma_start(
                            out=lanes[b][n % NLANES], in_=hc,
                            accum_op=(OP.bypass if n < NLANES else OP.add))
                # finish: y = (xc*Dp + sum(lanes)) * silu(z)
                for b in range(NB):
                    l01 = work.tile([128, LQ], BF16, tag="l01", name="l01", bufs=1)
                    nc.vector.tensor_tensor(l01, lanes[b][0], lanes[b][1], OP.add)
                    l23 = work.tile([128, LQ], BF16, tag="l23", name="l23", bufs=1)
                    nc.vector.tensor_tensor(l23, lanes[b][2], lanes[b][3], OP.add)
                    t1 = work.tile([128, LQ], BF16, tag="t1f", name="t1f", bufs=1)
                    nc.vector.scalar_tensor_tensor(
                        t1, xc[b][:, qsl], dp_sb[b][:, 0:1], l01, OP.mult, OP.add)
                    t2 = work.tile([128, LQ], BF16, tag="t2f", name="t2f", bufs=1)
                    nc.vector.tensor_tensor(t2, t1, l23, OP.add)
                    nc.vector.tensor_tensor(yfin[b][:, qsl], t2,
                                            zsil[b][:, qsl], OP.mult)

            # ---------- select / flip, W_out matmul, ReduceScatter ----------
            # ysel reuses the (dead) xs_pad slabs
            ysel = [persist.tile([128, L], BF16, tag=f"xsp{b}", name=f"ysel{b}")
                    for b in range(NB)]
            for b in range(NB):
                nc.vector.tensor_scalar(ysel[b], yfin[b], sel_sb[:, 0:1], None, OP.mult)
                nc.vector.scalar_tensor_tensor(
                    ysel[b], yfin[b][:, ::-1], sel_sb[:, 1:2], ysel[b], OP.mult, OP.add)
            for t in range(L // 128):
                tsl = slice(t * 128, (t + 1) * 128)
                ps = psum.tile([128, 512], FP32, tag="mm", name="mm")[:, 0:DM]
                for b in range(NB):
                    nc.tensor.matmul(ps, ysel[b][:, tsl], w_out_sb[b],
                                     start=(b == 0), stop=(b == NB - 1))
                yp = work.tile([128, DM], FP32, tag="ypart", name="ypart", bufs=3)
                nc.scalar.copy(yp, ps)
                nc.sync.dma_start(out=rs_in[tsl, :], in_=yp)
            nc.gpsimd.collective_compute(
                "ReduceScatter", OP.add,
                replica_groups=[[0, 1, 2, 3], [4, 5, 6, 7]],
                ins=[rs_in[:, :]], outs=[rs_out[:, :]])

            # ---------- phase 4: FFN + layernorm on the token slab ----------
            # YT via PE transpose (fp32), cast to bf16
            yT_bf = [work.tile([128, T_SLAB], BF16, tag=f"yT{k}", name=f"yT{k}")
                     for k in range(2)]
            for t8 in range(8):
                ysb = work.tile([128, DM], FP32, tag="ysb", name="ysb", bufs=2)
                nc.sync.dma_start(out=ysb, in_=rs_out[t8 * 128:(t8 + 1) * 128, :])
                for k in range(2):
                    pst = psum.tile([128, 512], FP32, tag="mm", name="mmtr")[:, 0:128]
                    nc.tensor.transpose(pst, ysb[:, k * 128:(k + 1) * 128], identity_bf)
                    nc.scalar.copy(yT_bf[k][:, t8 * 128:(t8 + 1) * 128], pst)
            # ff1 -> gelu, laid out (H, T); gelu tiles alias the xT slabs
            g_half = [persist.tile([128, L], BF16, tag=f"big{b}", name=f"gh{b}")
                      for b in range(2)]
            g_sb = [g_half[m // 4][:, (m % 4) * T_SLAB:(m % 4 + 1) * T_SLAB]
                    for m in range(8)]
            for m in range(8):
                ps = psumf.tile([128, T_SLAB], FP32, tag="mmf", name="mmf")
                for jj in range(2):
                    for k in range(2):
                        nc.tensor.matmul(
                            ps[:, jj * 512:(jj + 1) * 512],
                            w_ff1_sb[k][:, m * 128:(m + 1) * 128],
                            yT_bf[k][:, jj * 512:(jj + 1) * 512],
                            start=(k == 0), stop=(k == 1))
                nc.scalar.activation(g_sb[m], ps, AF.Gelu, bias=b_ff1_sb[:, m:m + 1])
            # ff2 + bias, then layernorm + residual
            for t in range(T_SLAB // 128):
                tsl = slice(t * 128, (t + 1) * 128)
                ps = psum.tile([128, 512], FP32, tag="mm", name="mm")[:, 0:DM]
                for m in range(8):
                    nc.tensor.matmul(ps, g_sb[m][:, tsl], w_ff2_sb[m],
                                     start=(m == 0), stop=(m == 7))
                ff = work.tile([128, DM], FP32, tag="ff", name="ff", bufs=2)
                nc.vector.tensor_tensor(ff, ps, b_ff2_sb, OP.add)
                stats = work.tile([128, 6], FP32, tag="stats", name="stats", bufs=2)
                nc.vector.bn_stats(stats, ff)
                mv = work.tile([128, 2], FP32, tag="mv", name="mv", bufs=2)
                nc.vector.bn_aggr(mv, stats)
                sq = work.tile([128, 1], FP32, tag="sq", name="sq", bufs=2)
                nc.scalar.activation(sq, mv[:, 1:2], AF.Sqrt, bias=eps_sb[:, 0:1])
                rstd = work.tile([128, 1], FP32, tag="rstd", name="rstd", bufs=2)
                nc.vector.reciprocal(rstd, sq)
                norm = work.tile([128, DM], FP32, tag="norm", name="norm", bufs=2)
                nc.vector.tensor_scalar(norm, ff, mv[:, 0:1], rstd,
                                        OP.subtract, OP.mult)
                nc.vector.tensor_tensor(norm, norm, g2_sb, OP.mult)
                nc.vector.tensor_tensor(norm, norm, beta2_sb, OP.add)
                ysb2 = work.tile([128, DM], FP32, tag="ysb", name="ysb2", bufs=2)
                nc.sync.dma_start(out=ysb2, in_=rs_out[tsl, :])
                nc.vector.tensor_tensor(norm, norm, ysb2, OP.add)
                nc.sync.dma_start(out=out_slab[tsl, :], in_=norm)

    nc.compile()
    return nc


def _prep_inputs(inputs):
    """Build the 8 per-core input maps from the full problem inputs."""
    x = np.asarray(inputs["x"], np.float32)
    W_in = np.asarray(inputs["W_in"], np.float32)
    conv_w = np.asarray(inputs["conv_w"], np.float32)
    conv_b = np.asarray(inputs["conv_b"], np.float32)
    W_xproj = np.asarray(inputs["W_xproj"], np.float32)
    W_dt = np.asarray(inputs["W_dt"], np.float32)
    b_dt = np.asarray(inputs["b_dt"], np.float32)
    A_log = np.asarray(inputs["A_log"], np.float32)
    Dp = np.asarray(inputs["Dp"], np.float32)
    W_out = np.asarray(inputs["W_out"], np.float32)
    W_ff1 = np.asarray(inputs["W_ff1"], np.float32)
    b_ff1 = np.asarray(inputs["b_ff1"], np.float32)
    W_ff2 = np.asarray(inputs["W_ff2"], np.float32)
    b_ff2 = np.asarray(inputs["b_ff2"], np.float32)
    g2 = np.asarray(inputs["g2"], np.float32)
    beta2 = np.asarray(inputs["beta2"], np.float32)

    A = -np.exp(A_log)  # (512, 16)
    in_maps = []
    for c in range(8):
        seq, half = c // 2, c % 2
        bb, bwd = seq // 2, seq % 2
        xs_np = x[bb]
        if bwd:
            xs_np = xs_np[::-1]
        hsl = slice(half * DI, (half + 1) * DI)
        m = dict(
            xT=np.ascontiguousarray(xs_np.T).astype(BF),
            w_xs=W_in[:, hsl].astype(BF),
            w_z=W_in[:, DI_FULL + half * DI: DI_FULL + (half + 1) * DI].astype(BF),
            conv_w=np.ascontiguousarray(conv_w[hsl]).astype(np.float32),
            conv_b=conv_b[hsl].reshape(DI, 1).astype(np.float32),
            w_xp=np.ascontiguousarray(W_xproj[hsl]).astype(BF),
            w_dt=np.ascontiguousarray(W_dt[:, hsl]).astype(BF),
            b_dt=b_dt[hsl].reshape(DI, 1).astype(np.float32),
            a_mat=np.ascontiguousarray(A[hsl]).astype(np.float32),
            dp=Dp[hsl].reshape(DI, 1).astype(np.float32),
            w_out=np.ascontiguousarray(W_out[hsl]).astype(BF),
            w_ff1=W_ff1.astype(BF),
            b_ff1=np.ascontiguousarray(b_ff1.reshape(H_FF // 128, 128).T).astype(np.float32),
            w_ff2=W_ff2.astype(BF),
            b_ff2r=np.broadcast_to(b_ff2, (128, DM)).astype(np.float32).copy(),
            g2r=np.broadcast_to(g2, (128, DM)).astype(np.float32).copy(),
            beta2r=np.broadcast_to(beta2, (128, DM)).astype(np.float32).copy(),
            sel=np.broadcast_to(
                np.array([1.0 - bwd, float(bwd)], np.float32), (128, 2)).copy(),
        )
        in_maps.append(m)
    return in_maps


def kernel(**inputs) -> np.ndarray:
    if "nc" not in _NC_CACHE:
        _NC_CACHE["nc"] = build_program()
    nc = _NC_CACHE["nc"]
    in_maps = _prep_inputs(inputs)
    res = run_bass_kernel_spmd(nc, in_maps, core_ids=list(range(8)))
    out = np.empty((B, L, DM), np.float32)
    for c in range(8):
        bb = c // 4
        r = c % 4
        out[bb, r * T_SLAB:(r + 1) * T_SLAB, :] = res.results[c]["out_slab"]
    return out

